# revision 7
# baseline (speedup 1.0000x reference)
"""Trainium2 Bass kernel for nn_DigitConvolutionalModel.

Model: out = relu(conv2d_valid(x.reshape(28,28), conv_w).reshape(676) @ w1 + b1) @ w2 + b2

Strategy (v3):
  - Conv folded into FC1 on host: W1' = C @ w1 [784, 300]; device runs a plain
    2-layer MLP. Pure data parallel over 8 NeuronCores: batch 65536 -> 8192.
  - Feature-major layout: host supplies x.T per core, prepacked into the exact
    per-pair SBUF tile order so every load is contiguous in DRAM. Only the 784
    real feature rows are transferred (the 112 pad rows of k-chunk 6 are
    memset once per persistent buffer); all loads are issued upfront on the
    single sync HWDGE ring in exact consumption order (x0k0, w1k0, x0k1,
    w1k1, ..., then pairs 1-7), so the SDMA engines stream at HBM rate and
    arrival order matches the k-major sweep. Output stores ride the scalar
    ring so they never queue behind x loads.
  - Layer 1 computes g = relu(W1'.T @ x.T + b1) as [300, batch]; k-major sweep
    with the 44-row m2 chunk col-tiled across two concurrent 64-wide column
    groups (batch halves j=0/j=1). K-padding to 896 costs no PE time (matmul
    streaming cost is N cycles regardless of K) -- only DMA, hence the no-pad
    transfers.
  - Layer 2 col-tiled 3-way: the three 128-row k-chunks of g contract in one
    PE slot per batch half (tile_position=(0,32*mi)), landing partials at psum
    partitions 0-9/32-41/64-73 of one bank; DVE folds the partials (+b2)
    reading the bank at three partition bases. 2 PE slots per pair instead of
    6. The final pair uses the chained (non-tiled) form instead: its drain is
    one ACT op + one DVE op in parallel, shortening the kernel tail.
  - PE warm-up matmuls cover the framework preamble -> first-data window so
    the HAM clock gate opens before real work; optional filler matmuls can
    bridge the pair-1 DMA ramp (FILL1) so the PE never idles >3.4us.
  - bf16 matmul inputs, fp32 PSUM accumulate.
"""

import numpy as np
import ml_dtypes

_B = 65536
_NCORES = 8
_BSH = _B // _NCORES  # 8192 batch rows per core
_N = 512  # batch columns per matmul (one fp32 PSUM bank)
_KP = 896  # padded input features (784 -> 7 chunks of 128)
_MP = 384  # padded hidden features (300 -> 3 chunks of 128)
_NK = _KP // 128  # 7
_NM = _MP // 128  # 3
_NPAIR = _BSH // (2 * _N)  # 8 pairs of 512-col batch tiles
_M2 = 300 - 256  # 44

_WARMUP = 26  # N=128 warm-up matmuls (cover preamble -> first data)
_FILL1 = 2   # N=512 filler matmuls between pair 0 and pair 1 (DMA ramp)

_state = {}


def _build_nc():
    import concourse.tile as tile
    from concourse import bacc, mybir
    from contextlib import ExitStack

    dt = mybir.dt
    AF = mybir.ActivationFunctionType

    nc = bacc.Bacc(
        "TRN2",
        target_bir_lowering=False,
        debug=False,
        enable_asserts=False,
        num_devices=_NCORES,
    )

    # x prepacked on host: xa = k-chunks 0-5 [pair, partition, ki, col]
    # (contiguous 1.57 MB per pair), xb = the 16 real rows of chunk 6.
    xa = nc.dram_tensor(
        "xa", [_NPAIR, 128, 6, 2 * _N], dt.bfloat16, kind="ExternalInput"
    ).ap()
    # 16 real rows of chunk 6 + 16 host-side zero rows (32-row transfer so the
    # SBUF pad memset can start at the 32-aligned partition base).
    xb = nc.dram_tensor(
        "xb", [_NPAIR, 32, 2 * _N], dt.bfloat16, kind="ExternalInput"
    ).ap()
    # w1 packed chunk-major per partition: [p, ki*384+m] = W1'[128ki+p, m]
    w1 = nc.dram_tensor("w1", [128, _NK * _MP], dt.bfloat16, kind="ExternalInput").ap()
    b1 = nc.dram_tensor("b1", [_MP, 1], dt.float32, kind="ExternalInput").ap()
    w2 = nc.dram_tensor("w2", [_MP, 10], dt.bfloat16, kind="ExternalInput").ap()
    b2 = nc.dram_tensor("b2", [10, 1], dt.float32, kind="ExternalInput").ap()
    outT = nc.dram_tensor("outT", [10, _BSH], dt.float32, kind="ExternalOutput").ap()

    b1_r = b1.rearrange("(m p) one -> p m one", p=128)  # [128, 3, 1]
    w2_r = w2.rearrange("(m p) o -> p m o", p=128)  # [128, 3, 10]
    w1_r = w1.rearrange("p (k m) -> p k m", k=_NK)  # [128, 7, 384]

    with tile.TileContext(nc) as tc, ExitStack() as ctx:
        wpool = ctx.enter_context(tc.tile_pool(name="wpool", bufs=1))
        xpool = ctx.enter_context(tc.tile_pool(name="xpool", bufs=1))
        gpool = ctx.enter_context(tc.tile_pool(name="gpool", bufs=2))
        ppool = ctx.enter_context(tc.tile_pool(name="ppool", bufs=5, space="PSUM"))
        pm2pool = ctx.enter_context(tc.tile_pool(name="pm2pool", bufs=1, space="PSUM"))
        p2pool = ctx.enter_context(tc.tile_pool(name="p2pool", bufs=2, space="PSUM"))
        opool = ctx.enter_context(tc.tile_pool(name="opool", bufs=2))

        # PE warm-up: dependency-free matmuls keep the TensorEngine busy from
        # the moment its preamble ends, so the HAM clock gate opens (~3.4us of
        # sustained activity) before the first real matmul issues.
        warm_in = wpool.tile([128, 128], dt.bfloat16, name="warm_in", tag="warm_in")
        nc.gpsimd.memset(warm_in[:], 0.0)
        warm_ps = p2pool.tile([128, 128], dt.float32, name="warm_ps", tag="ps2")
        for _ in range(_WARMUP):
            nc.tensor.matmul(
                out=warm_ps[:], lhsT=warm_in[:], rhs=warm_in[:], start=True, stop=True
            )

        # Persistent x buffers; the chunk-6 pad partitions (16-127) are memset
        # once per buffer (both the w1 pad rows and these must be finite so
        # 0-weight x 0-value stays 0, never NaN).
        xts = [
            xpool.tile([128, _NK, 2 * _N], dt.bfloat16, name=f"xt_{p}", tag=f"xt{p}")
            for p in range(_NPAIR)
        ]
        for p in range(_NPAIR):
            nc.gpsimd.memset(xts[p][32:64, 6, :], 0.0)
            nc.gpsimd.memset(xts[p][64:128, 6, :], 0.0)

        # All loads upfront on the sync ring in exact consumption order.
        w1sb = wpool.tile([128, _NK, _MP], dt.bfloat16, name="w1sb", tag="w1sb")
        for ki in range(6):
            nc.sync.dma_start(out=xts[0][:, ki, :], in_=xa[0, :, ki, :])
            nc.sync.dma_start(out=w1sb[:, ki, :], in_=w1_r[:, ki, :])
        nc.sync.dma_start(out=xts[0][0:32, 6, :], in_=xb[0, :, :])
        nc.sync.dma_start(out=w1sb[:, 6, :], in_=w1_r[:, 6, :])
        b1sb = wpool.tile([128, _NM, 1], dt.float32, name="b1sb", tag="b1sb")
        nc.sync.dma_start(out=b1sb[:], in_=b1_r[:])
        w2sb = wpool.tile([128, _NM, 10], dt.bfloat16, name="w2sb", tag="w2sb")
        nc.sync.dma_start(out=w2sb[:], in_=w2_r[:])
        b2sb = wpool.tile([10, 1], dt.float32, name="b2sb", tag="b2sb")
        nc.sync.dma_start(out=b2sb[:], in_=b2[:, :])
        # m2 chunk (44 real rows): batch half j=1 lands at partitions 64.. so
        # its bias and layer-2 weights need partition-64-aligned replicas.
        b1rep = wpool.tile([128, 1], dt.float32, name="b1rep", tag="b1rep")
        nc.sync.dma_start(out=b1rep[64 : 64 + _M2, :], in_=b1_r[0:_M2, 2, :])
        w2rep = wpool.tile([128, 10], dt.bfloat16, name="w2rep", tag="w2rep")
        nc.gpsimd.memset(w2rep[:], 0.0)
        nc.sync.dma_start(out=w2rep[64 : 64 + _M2, :], in_=w2_r[0:_M2, 2, :])
        for p in range(1, _NPAIR):
            nc.sync.dma_start(out=xts[p][:, 0:6, :], in_=xa[p, :, :, :])
            nc.sync.dma_start(out=xts[p][0:32, 6, :], in_=xb[p, :, :])

        def layer2_tiled(prev_g, prev_c0):
            """Col-tiled second layer: per batch half, the three k-chunks of g
            contract concurrently in col groups 0/32/64 of one PE slot; DVE
            folds the three psum partials (+ b2); result stores on the scalar
            ring."""
            for j in range(2):
                ps2 = p2pool.tile([128, _N], dt.float32, name=f"ps2_{prev_c0}_{j}", tag="ps2")
                for mi in range(_NM):
                    lhsT = w2sb[:, mi, :]
                    if mi == 2 and j == 1:
                        lhsT = w2rep[:]
                    nc.tensor.matmul(
                        out=ps2[32 * mi : 32 * mi + 10, :],
                        lhsT=lhsT,
                        rhs=prev_g[(mi, j)][:],
                        start=True,
                        stop=True,
                        tile_position=(0, 32 * mi),
                    )
                t1 = opool.tile([10, _N], dt.float32, name=f"t1_{prev_c0}_{j}", tag="t1")
                nc.vector.tensor_scalar(
                    t1[:], ps2[0:10, :], b2sb[:], None, mybir.AluOpType.add
                )
                t2 = opool.tile([10, _N], dt.float32, name=f"t2_{prev_c0}_{j}", tag="t2")
                nc.vector.tensor_tensor(
                    t2[:], t1[:], ps2[32:42, :], mybir.AluOpType.add
                )
                ob = opool.tile([10, _N], dt.float32, name=f"ob_{prev_c0}_{j}", tag="ob")
                nc.vector.tensor_tensor(
                    ob[:], t2[:], ps2[64:74, :], mybir.AluOpType.add
                )
                c0 = prev_c0 + j * _N
                nc.scalar.dma_start(out=outT[:, c0 : c0 + _N], in_=ob[:])

        def layer2_chained(prev_g, prev_c0):
            """Chained second layer (final pair): 3 accumulating matmuls per
            batch half into a [10, N] psum; drain is a single ACT (j=0) or DVE
            (j=1) op, so the two halves drain on parallel engines."""
            for j in range(2):
                ps2 = p2pool.tile([10, _N], dt.float32, name=f"ps2f_{j}", tag="ps2")
                for mi in range(_NM):
                    lhsT = w2sb[:, mi, :]
                    if mi == 2 and j == 1:
                        lhsT = w2rep[:]
                    nc.tensor.matmul(
                        out=ps2[:],
                        lhsT=lhsT,
                        rhs=prev_g[(mi, j)][:],
                        start=(mi == 0),
                        stop=(mi == _NM - 1),
                    )
                ob = opool.tile([10, _N], dt.float32, name=f"obf_{j}", tag="ob")
                if j == 0:
                    nc.scalar.activation(
                        ob[:], ps2[:], AF.Identity, bias=b2sb[:], scale=1.0
                    )
                else:
                    nc.vector.tensor_scalar(
                        ob[:], ps2[:], b2sb[:], None, mybir.AluOpType.add
                    )
                c0 = prev_c0 + j * _N
                nc.scalar.dma_start(out=outT[:, c0 : c0 + _N], in_=ob[:])

        prev_g = None
        prev_c0 = 0
        for pair in range(_NPAIR):
            c0 = pair * 2 * _N
            xtile = xts[pair]
            if pair == 1:
                for f in range(_FILL1):
                    fl = p2pool.tile([128, _N], dt.float32, name=f"fl1_{f}", tag="ps2")
                    nc.tensor.matmul(
                        out=fl[:, 0:128], lhsT=warm_in[:], rhs=warm_in[:],
                        start=True, stop=True,
                    )

            cur_g = {}
            # k-major sweep: consume each x-chunk for all four (mi, j)
            # accumulators before needing the next chunk.
            ps = {
                (mi, j): ppool.tile(
                    [128, _N], dt.float32, name=f"ps_{pair}_{mi}_{j}", tag="ps"
                )
                for mi in range(2)
                for j in range(2)
            }
            for ki in range(_NK):
                for mi in range(2):
                    for j in range(2):
                        nc.tensor.matmul(
                            out=ps[(mi, j)][:],
                            lhsT=w1sb[:, ki, mi * 128 : (mi + 1) * 128],
                            rhs=xtile[:, ki, j * _N : (j + 1) * _N],
                            start=(ki == 0),
                            stop=(ki == _NK - 1),
                        )
                if ki == 3 and prev_g is not None:
                    # Software-pipelined layer 2 for the previous pair.
                    layer2_tiled(prev_g, prev_c0)
            for mi in range(2):
                for j in range(2):
                    g = gpool.tile(
                        [128, _N], dt.bfloat16, name=f"g_{pair}_{mi}_{j}", tag=f"g{mi}{j}"
                    )
                    if j == 0:
                        # Split the relus across ACT and DVE so neither engine
                        # serializes the psum drain.
                        nc.scalar.activation(
                            g[:], ps[(mi, j)][:], AF.Relu, bias=b1sb[:, mi, :], scale=1.0
                        )
                    else:
                        nc.vector.tensor_scalar(
                            g[:], ps[(mi, j)][:], b1sb[:, mi, :], 0.0,
                            mybir.AluOpType.add, mybir.AluOpType.max,
                        )
                    cur_g[(mi, j)] = g

            # m2 chunk (44 output rows): both batch halves run concurrently as
            # col-tiled matmuls — j=0 writes psum partitions 0..43 (col group
            # 0), j=1 writes partitions 64..107 (col group 64) of one bank.
            psm2 = pm2pool.tile([128, _N], dt.float32, name=f"psm2_{pair}", tag="psm2")
            for ki in range(_NK):
                for j in range(2):
                    nc.tensor.matmul(
                        out=psm2[64 * j : 64 * j + _M2, :],
                        lhsT=w1sb[:, ki, 256 : 256 + _M2],
                        rhs=xtile[:, ki, j * _N : (j + 1) * _N],
                        start=(ki == 0),
                        stop=(ki == _NK - 1),
                        tile_position=(0, 64 * j),
                    )
            # g tiles are full 128 rows with the unused rows zeroed so layer 2
            # can use uniform full-row matmuls (0-weight x 0-value, never NaN).
            g20 = gpool.tile([128, _N], dt.bfloat16, name=f"g_{pair}_2_0", tag="g20")
            nc.gpsimd.memset(g20[32:64, :], 0.0)  # 32-aligned; relu rewrites 32..43
            nc.gpsimd.memset(g20[64:128, :], 0.0)
            nc.scalar.activation(
                g20[0:_M2, :], psm2[0:_M2, :], AF.Relu, bias=b1sb[0:_M2, 2, :], scale=1.0
            )
            g21 = gpool.tile([128, _N], dt.bfloat16, name=f"g_{pair}_2_1", tag="g21")
            nc.gpsimd.memset(g21[0:64, :], 0.0)
            nc.gpsimd.memset(g21[96:128, :], 0.0)  # 32-aligned; relu rewrites 96..107
            nc.vector.tensor_scalar(
                g21[64 : 64 + _M2, :], psm2[64 : 64 + _M2, :], b1rep[64 : 64 + _M2, :],
                0.0, mybir.AluOpType.add, mybir.AluOpType.max,
            )
            cur_g[(2, 0)] = g20
            cur_g[(2, 1)] = g21
            prev_g = cur_g
            prev_c0 = c0
        layer2_chained(prev_g, prev_c0)

    nc.compile()
    return nc


def _fold_conv(conv_w, w1):
    """W1' = C @ w1 where C [784, 676] is the linear map of the 3x3 valid conv."""
    C = np.zeros((784, 676), np.float64)
    cw = np.asarray(conv_w, np.float64)
    for di in range(3):
        for dj in range(3):
            for i in range(26):
                rows = (i + di) * 28 + dj + np.arange(26)
                C[rows, i * 26 + np.arange(26)] += cw[di, dj]
    return C @ np.asarray(w1, np.float64)  # [784, 300]


def _exec(inputs, trace=False, **run_kwargs):
    from concourse.bass_utils import run_bass_kernel_spmd

    x = np.asarray(inputs["x"], np.float32)
    bf16 = ml_dtypes.bfloat16

    w1f = np.zeros((_KP, _MP), bf16)
    w1f[:784, :300] = _fold_conv(inputs["conv_w"], inputs["w1"]).astype(bf16)
    # chunk-major per partition: [p, ki*384+m] = W1'[128ki+p, m]
    w1pk = np.ascontiguousarray(
        w1f.reshape(_NK, 128, _MP).transpose(1, 0, 2).reshape(128, _NK * _MP)
    )
    b1c = np.zeros((_MP, 1), np.float32)
    b1c[:300, 0] = np.asarray(inputs["b1"], np.float32)
    w2b = np.zeros((_MP, 10), bf16)
    w2b[:300] = np.asarray(inputs["w2"], np.float32).astype(bf16)
    b2c = np.ascontiguousarray(np.asarray(inputs["b2"], np.float32).reshape(10, 1))

    if "nc" not in _state:
        _state["nc"] = _build_nc()
    nc = _state["nc"]

    xb16 = x.astype(bf16)  # [65536, 784]
    in_maps = []
    for c in range(_NCORES):
        xT = xb16[c * _BSH : (c + 1) * _BSH, :].T  # [784, 8192]
        xav = np.ascontiguousarray(
            xT[:768].reshape(6, 128, _NPAIR, 2 * _N).transpose(2, 1, 0, 3)
        )
        xTp = np.zeros((32, _BSH), bf16)
        xTp[:16] = xT[768:784]
        xbv = np.ascontiguousarray(
            xTp.reshape(32, _NPAIR, 2 * _N).transpose(1, 0, 2)
        )
        in_maps.append(
            {"xa": xav, "xb": xbv, "w1": w1pk, "b1": b1c, "w2": w2b, "b2": b2c}
        )

    res = run_bass_kernel_spmd(
        nc, in_maps, list(range(_NCORES)), trace=trace, **run_kwargs
    )
    outs = [res.results[c]["outT"] for c in range(_NCORES)]  # each [10, 8192]
    out = np.concatenate(outs, axis=1).T  # [65536, 10]
    return np.ascontiguousarray(out, dtype=np.float32), res


def kernel(**inputs):
    out, _ = _exec(inputs, trace=False)
    return out


# revision 8
# speedup vs baseline: 1.2058x; 1.2058x over previous
"""Trainium2 Bass kernel for nn_DigitConvolutionalModel.

Model: out = relu(conv2d_valid(x.reshape(28,28), conv_w).reshape(676) @ w1 + b1) @ w2 + b2

Strategy (v3):
  - Conv folded into FC1 on host: W1' = C @ w1 [784, 300]; device runs a plain
    2-layer MLP. Pure data parallel over 8 NeuronCores: batch 65536 -> 8192.
  - Feature-major layout: host supplies x.T per core, prepacked into the exact
    per-pair SBUF tile order so every load is contiguous in DRAM. Only the 784
    real feature rows are transferred (the 112 pad rows of k-chunk 6 are
    memset once per persistent buffer); all loads are issued upfront on the
    single sync HWDGE ring in exact consumption order (x0k0, w1k0, x0k1,
    w1k1, ..., then pairs 1-7), so the SDMA engines stream at HBM rate and
    arrival order matches the k-major sweep. Output stores ride the scalar
    ring so they never queue behind x loads.
  - Layer 1 computes g = relu(W1'.T @ x.T + b1) as [300, batch]; k-major sweep
    with the 44-row m2 chunk col-tiled across two concurrent 64-wide column
    groups (batch halves j=0/j=1). K-padding to 896 costs no PE time (matmul
    streaming cost is N cycles regardless of K) -- only DMA, hence the no-pad
    transfers.
  - Layer 2 col-tiled 3-way: the three 128-row k-chunks of g contract in one
    PE slot per batch half (tile_position=(0,32*mi)), landing partials at psum
    partitions 0-9/32-41/64-73 of one bank; DVE folds the partials (+b2)
    reading the bank at three partition bases. 2 PE slots per pair instead of
    6. The final pair uses the chained (non-tiled) form instead: its drain is
    one ACT op + one DVE op in parallel, shortening the kernel tail.
  - PE warm-up matmuls cover the framework preamble -> first-data window so
    the HAM clock gate opens before real work; optional filler matmuls can
    bridge the pair-1 DMA ramp (FILL1) so the PE never idles >3.4us.
  - bf16 matmul inputs, fp32 PSUM accumulate.
"""

import numpy as np
import ml_dtypes

_B = 65536
_NCORES = 8
_BSH = _B // _NCORES  # 8192 batch rows per core
_N = 512  # batch columns per matmul (one fp32 PSUM bank)
_KP = 896  # padded input features (784 -> 7 chunks of 128)
_MP = 384  # padded hidden features (300 -> 3 chunks of 128)
_NK = _KP // 128  # 7
_NM = _MP // 128  # 3
_NPAIR = _BSH // (2 * _N)  # 8 pairs of 512-col batch tiles
_M2 = 300 - 256  # 44

_WARMUP = 26  # N=128 warm-up matmuls (cover preamble -> first data)
_FILL1 = 4   # N=512 filler matmuls between pair 0 and pair 1 (DMA ramp)

_state = {}


def _build_nc():
    import concourse.tile as tile
    from concourse import bacc, mybir
    from contextlib import ExitStack

    dt = mybir.dt
    AF = mybir.ActivationFunctionType

    nc = bacc.Bacc(
        "TRN2",
        target_bir_lowering=False,
        debug=False,
        enable_asserts=False,
        num_devices=_NCORES,
    )

    # x prepacked on host as [pair, partition, ki, col] incl. zero-padded
    # chunk-6 rows: each pair slice is one fully-contiguous 1.79 MB DMA.
    # (dma_start issue costs ~0.7us of engine time each and in-flight DMAs
    # are capped by 8 sem lanes, so few big transfers beat many small ones.)
    xt = nc.dram_tensor(
        "xt", [_NPAIR, 128, _NK, 2 * _N], dt.bfloat16, kind="ExternalInput"
    ).ap()
    # w1 packed chunk-major per partition: [p, ki*384+m] = W1'[128ki+p, m]
    w1 = nc.dram_tensor("w1", [128, _NK * _MP], dt.bfloat16, kind="ExternalInput").ap()
    b1 = nc.dram_tensor("b1", [_MP, 1], dt.float32, kind="ExternalInput").ap()
    w2 = nc.dram_tensor("w2", [_MP, 10], dt.bfloat16, kind="ExternalInput").ap()
    b2 = nc.dram_tensor("b2", [10, 1], dt.float32, kind="ExternalInput").ap()
    outT = nc.dram_tensor("outT", [10, _BSH], dt.float32, kind="ExternalOutput").ap()

    b1_r = b1.rearrange("(m p) one -> p m one", p=128)  # [128, 3, 1]
    w2_r = w2.rearrange("(m p) o -> p m o", p=128)  # [128, 3, 10]
    w1_r = w1.rearrange("p (k m) -> p k m", k=_NK)  # [128, 7, 384]

    with tile.TileContext(nc) as tc, ExitStack() as ctx:
        wpool = ctx.enter_context(tc.tile_pool(name="wpool", bufs=1))
        xpool = ctx.enter_context(tc.tile_pool(name="xpool", bufs=1))
        gpool = ctx.enter_context(tc.tile_pool(name="gpool", bufs=2))
        ppool = ctx.enter_context(tc.tile_pool(name="ppool", bufs=5, space="PSUM"))
        pm2pool = ctx.enter_context(tc.tile_pool(name="pm2pool", bufs=1, space="PSUM"))
        p2pool = ctx.enter_context(tc.tile_pool(name="p2pool", bufs=2, space="PSUM"))
        opool = ctx.enter_context(tc.tile_pool(name="opool", bufs=2))

        # PE warm-up: dependency-free matmuls keep the TensorEngine busy from
        # the moment its preamble ends, so the HAM clock gate opens (~3.4us of
        # sustained activity) before the first real matmul issues.
        warm_in = wpool.tile([128, 128], dt.bfloat16, name="warm_in", tag="warm_in")
        nc.gpsimd.memset(warm_in[:], 0.0)
        warm_ps = p2pool.tile([128, 128], dt.float32, name="warm_ps", tag="ps2")
        for _ in range(_WARMUP):
            nc.tensor.matmul(
                out=warm_ps[:], lhsT=warm_in[:], rhs=warm_in[:], start=True, stop=True
            )

        # Persistent x buffers; the chunk-6 pad partitions (16-127) are memset
        # once per buffer (both the w1 pad rows and these must be finite so
        # 0-weight x 0-value stays 0, never NaN).
        xts = [
            xpool.tile([128, _NK, 2 * _N], dt.bfloat16, name=f"xt_{p}", tag=f"xt{p}")
            for p in range(_NPAIR)
        ]
        # Loads upfront, split across three issue paths: x on the sync HWDGE
        # ring (pair 0 in three pieces so the first matmul waits on only 0.5
        # MB), w1 as two contiguous slabs on the scalar HWDGE ring (streams in
        # parallel with pair-0 x via SDMA packet round-robin), and the small
        # weights via gpsimd/SWDGE so they cost no HWDGE ring time at all.
        w1sb = wpool.tile([128, _NK, _MP], dt.bfloat16, name="w1sb", tag="w1sb")
        nc.sync.dma_start(out=xts[0][:, 0:2, :], in_=xt[0, :, 0:2, :])
        nc.scalar.dma_start(out=w1sb[:, 0:2, :], in_=w1_r[:, 0:2, :])
        nc.sync.dma_start(out=xts[0][:, 2:4, :], in_=xt[0, :, 2:4, :])
        nc.scalar.dma_start(out=w1sb[:, 2:7, :], in_=w1_r[:, 2:7, :])
        nc.sync.dma_start(out=xts[0][:, 4:7, :], in_=xt[0, :, 4:7, :])
        for p in range(1, _NPAIR):
            nc.sync.dma_start(out=xts[p][:], in_=xt[p, :, :, :])
        b1sb = wpool.tile([128, _NM, 1], dt.float32, name="b1sb", tag="b1sb")
        nc.gpsimd.dma_start(out=b1sb[:], in_=b1_r[:])
        w2sb = wpool.tile([128, _NM, 10], dt.bfloat16, name="w2sb", tag="w2sb")
        nc.gpsimd.dma_start(out=w2sb[:], in_=w2_r[:])
        b2sb = wpool.tile([10, 1], dt.float32, name="b2sb", tag="b2sb")
        nc.gpsimd.dma_start(out=b2sb[:], in_=b2[:, :])
        # m2 chunk (44 real rows): batch half j=1 lands at partitions 64.. so
        # its bias and layer-2 weights need partition-64-aligned replicas.
        b1rep = wpool.tile([128, 1], dt.float32, name="b1rep", tag="b1rep")
        nc.gpsimd.dma_start(out=b1rep[64 : 64 + _M2, :], in_=b1_r[0:_M2, 2, :])
        w2rep = wpool.tile([128, 10], dt.bfloat16, name="w2rep", tag="w2rep")
        nc.gpsimd.memset(w2rep[:], 0.0)
        nc.gpsimd.dma_start(out=w2rep[64 : 64 + _M2, :], in_=w2_r[0:_M2, 2, :])

        def layer2_tiled(prev_g, prev_c0):
            """Col-tiled second layer: per batch half, the three k-chunks of g
            contract concurrently in col groups 0/32/64 of one PE slot; DVE
            folds the three psum partials (+ b2); result stores on the scalar
            ring."""
            for j in range(2):
                ps2 = p2pool.tile([128, _N], dt.float32, name=f"ps2_{prev_c0}_{j}", tag="ps2")
                for mi in range(_NM):
                    lhsT = w2sb[:, mi, :]
                    if mi == 2 and j == 1:
                        lhsT = w2rep[:]
                    nc.tensor.matmul(
                        out=ps2[32 * mi : 32 * mi + 10, :],
                        lhsT=lhsT,
                        rhs=prev_g[(mi, j)][:],
                        start=True,
                        stop=True,
                        tile_position=(0, 32 * mi),
                    )
                t1 = opool.tile([10, _N], dt.float32, name=f"t1_{prev_c0}_{j}", tag="t1")
                nc.vector.tensor_scalar(
                    t1[:], ps2[0:10, :], b2sb[:], None, mybir.AluOpType.add
                )
                t2 = opool.tile([10, _N], dt.float32, name=f"t2_{prev_c0}_{j}", tag="t2")
                nc.vector.tensor_tensor(
                    t2[:], t1[:], ps2[32:42, :], mybir.AluOpType.add
                )
                ob = opool.tile([10, _N], dt.float32, name=f"ob_{prev_c0}_{j}", tag="ob")
                nc.vector.tensor_tensor(
                    ob[:], t2[:], ps2[64:74, :], mybir.AluOpType.add
                )
                c0 = prev_c0 + j * _N
                nc.scalar.dma_start(out=outT[:, c0 : c0 + _N], in_=ob[:])

        def layer2_chained(prev_g, prev_c0):
            """Chained second layer (final pair): 3 accumulating matmuls per
            batch half into a [10, N] psum; drain is a single ACT (j=0) or DVE
            (j=1) op, so the two halves drain on parallel engines."""
            for j in range(2):
                ps2 = p2pool.tile([10, _N], dt.float32, name=f"ps2f_{j}", tag="ps2")
                for mi in range(_NM):
                    lhsT = w2sb[:, mi, :]
                    if mi == 2 and j == 1:
                        lhsT = w2rep[:]
                    nc.tensor.matmul(
                        out=ps2[:],
                        lhsT=lhsT,
                        rhs=prev_g[(mi, j)][:],
                        start=(mi == 0),
                        stop=(mi == _NM - 1),
                    )
                ob = opool.tile([10, _N], dt.float32, name=f"obf_{j}", tag="ob")
                if j == 0:
                    nc.scalar.activation(
                        ob[:], ps2[:], AF.Identity, bias=b2sb[:], scale=1.0
                    )
                else:
                    nc.vector.tensor_scalar(
                        ob[:], ps2[:], b2sb[:], None, mybir.AluOpType.add
                    )
                c0 = prev_c0 + j * _N
                nc.scalar.dma_start(out=outT[:, c0 : c0 + _N], in_=ob[:])

        prev_g = None
        prev_c0 = 0
        for pair in range(_NPAIR):
            c0 = pair * 2 * _N
            xtile = xts[pair]
            if pair == 1:
                for f in range(_FILL1):
                    fl = p2pool.tile([128, _N], dt.float32, name=f"fl1_{f}", tag="ps2")
                    nc.tensor.matmul(
                        out=fl[:, 0:128], lhsT=warm_in[:], rhs=warm_in[:],
                        start=True, stop=True,
                    )

            cur_g = {}
            # k-major sweep: consume each x-chunk for all four (mi, j)
            # accumulators before needing the next chunk.
            ps = {
                (mi, j): ppool.tile(
                    [128, _N], dt.float32, name=f"ps_{pair}_{mi}_{j}", tag="ps"
                )
                for mi in range(2)
                for j in range(2)
            }
            for ki in range(_NK):
                for mi in range(2):
                    for j in range(2):
                        nc.tensor.matmul(
                            out=ps[(mi, j)][:],
                            lhsT=w1sb[:, ki, mi * 128 : (mi + 1) * 128],
                            rhs=xtile[:, ki, j * _N : (j + 1) * _N],
                            start=(ki == 0),
                            stop=(ki == _NK - 1),
                        )
                if ki == 3 and prev_g is not None:
                    # Software-pipelined layer 2 for the previous pair.
                    layer2_tiled(prev_g, prev_c0)
            for mi in range(2):
                for j in range(2):
                    g = gpool.tile(
                        [128, _N], dt.bfloat16, name=f"g_{pair}_{mi}_{j}", tag=f"g{mi}{j}"
                    )
                    if j == 0:
                        # Split the relus across ACT and DVE so neither engine
                        # serializes the psum drain.
                        nc.scalar.activation(
                            g[:], ps[(mi, j)][:], AF.Relu, bias=b1sb[:, mi, :], scale=1.0
                        )
                    else:
                        nc.vector.tensor_scalar(
                            g[:], ps[(mi, j)][:], b1sb[:, mi, :], 0.0,
                            mybir.AluOpType.add, mybir.AluOpType.max,
                        )
                    cur_g[(mi, j)] = g

            # m2 chunk (44 output rows): both batch halves run concurrently as
            # col-tiled matmuls — j=0 writes psum partitions 0..43 (col group
            # 0), j=1 writes partitions 64..107 (col group 64) of one bank.
            psm2 = pm2pool.tile([128, _N], dt.float32, name=f"psm2_{pair}", tag="psm2")
            for ki in range(_NK):
                for j in range(2):
                    nc.tensor.matmul(
                        out=psm2[64 * j : 64 * j + _M2, :],
                        lhsT=w1sb[:, ki, 256 : 256 + _M2],
                        rhs=xtile[:, ki, j * _N : (j + 1) * _N],
                        start=(ki == 0),
                        stop=(ki == _NK - 1),
                        tile_position=(0, 64 * j),
                    )
            # g tiles are full 128 rows with the unused rows zeroed so layer 2
            # can use uniform full-row matmuls (0-weight x 0-value, never NaN).
            g20 = gpool.tile([128, _N], dt.bfloat16, name=f"g_{pair}_2_0", tag="g20")
            nc.gpsimd.memset(g20[32:64, :], 0.0)  # 32-aligned; relu rewrites 32..43
            nc.gpsimd.memset(g20[64:128, :], 0.0)
            nc.scalar.activation(
                g20[0:_M2, :], psm2[0:_M2, :], AF.Relu, bias=b1sb[0:_M2, 2, :], scale=1.0
            )
            g21 = gpool.tile([128, _N], dt.bfloat16, name=f"g_{pair}_2_1", tag="g21")
            nc.gpsimd.memset(g21[0:64, :], 0.0)
            nc.gpsimd.memset(g21[96:128, :], 0.0)  # 32-aligned; relu rewrites 96..107
            nc.vector.tensor_scalar(
                g21[64 : 64 + _M2, :], psm2[64 : 64 + _M2, :], b1rep[64 : 64 + _M2, :],
                0.0, mybir.AluOpType.add, mybir.AluOpType.max,
            )
            cur_g[(2, 0)] = g20
            cur_g[(2, 1)] = g21
            prev_g = cur_g
            prev_c0 = c0
        layer2_chained(prev_g, prev_c0)

    nc.compile()
    return nc


def _fold_conv(conv_w, w1):
    """W1' = C @ w1 where C [784, 676] is the linear map of the 3x3 valid conv."""
    C = np.zeros((784, 676), np.float64)
    cw = np.asarray(conv_w, np.float64)
    for di in range(3):
        for dj in range(3):
            for i in range(26):
                rows = (i + di) * 28 + dj + np.arange(26)
                C[rows, i * 26 + np.arange(26)] += cw[di, dj]
    return C @ np.asarray(w1, np.float64)  # [784, 300]


def _exec(inputs, trace=False, **run_kwargs):
    from concourse.bass_utils import run_bass_kernel_spmd

    x = np.asarray(inputs["x"], np.float32)
    bf16 = ml_dtypes.bfloat16

    w1f = np.zeros((_KP, _MP), bf16)
    w1f[:784, :300] = _fold_conv(inputs["conv_w"], inputs["w1"]).astype(bf16)
    # chunk-major per partition: [p, ki*384+m] = W1'[128ki+p, m]
    w1pk = np.ascontiguousarray(
        w1f.reshape(_NK, 128, _MP).transpose(1, 0, 2).reshape(128, _NK * _MP)
    )
    b1c = np.zeros((_MP, 1), np.float32)
    b1c[:300, 0] = np.asarray(inputs["b1"], np.float32)
    w2b = np.zeros((_MP, 10), bf16)
    w2b[:300] = np.asarray(inputs["w2"], np.float32).astype(bf16)
    b2c = np.ascontiguousarray(np.asarray(inputs["b2"], np.float32).reshape(10, 1))

    if "nc" not in _state:
        _state["nc"] = _build_nc()
    nc = _state["nc"]

    xb16 = x.astype(bf16)  # [65536, 784]
    in_maps = []
    for c in range(_NCORES):
        xT = xb16[c * _BSH : (c + 1) * _BSH, :].T  # [784, 8192]
        xTp = np.zeros((_KP, _BSH), bf16)
        xTp[:784] = xT
        pk = np.ascontiguousarray(
            xTp.reshape(_NK, 128, _NPAIR, 2 * _N).transpose(2, 1, 0, 3)
        )
        in_maps.append(
            {"xt": pk, "w1": w1pk, "b1": b1c, "w2": w2b, "b2": b2c}
        )

    res = run_bass_kernel_spmd(
        nc, in_maps, list(range(_NCORES)), trace=trace, **run_kwargs
    )
    outs = [res.results[c]["outT"] for c in range(_NCORES)]  # each [10, 8192]
    out = np.concatenate(outs, axis=1).T  # [65536, 10]
    return np.ascontiguousarray(out, dtype=np.float32), res


def kernel(**inputs):
    out, _ = _exec(inputs, trace=False)
    return out
